# revision 3
# baseline (speedup 1.0000x reference)
"""AWQ int8 linear (x @ (W_q * scale).T + bias) on 8 Trainium2 NeuronCores.

Column-parallel sharding: out_features padded 11008 -> 11264 and split 1408
per core; x is replicated. Per core computes outT[o, t] = scale[o] *
sum_i W_q[o, i] * x[t, i] + bias[o], PE accumulating in fp32 PSUM, with a
per-partition scale+bias fused into one ScalarE activation.

Two precision variants:
 - "fp32r": x kept as fp32 (PE fp32r mode, ~1e-4 rel err). x is too big for
   SBUF in fp32, so the token dim is processed in 2 resident halves and the
   weights are streamed twice (int8->fp32r cast in the DMA).
 - "bf16": x host-cast to bf16 (~1.7e-3 rel err), whole x resident, weights
   streamed once (int8->bf16 cast in the DMA).

Host-side layout prep: x is transposed to xT [IN_F, T] so the contraction
dim lands on SBUF partitions; per-core weights are pre-arranged to the exact
SBUF image [ki=128, m, ko, o] so every DMA is contiguous per partition.
The first m-tiles of each half are processed k-interleaved as a group so the
PE has enough work per arriving x k-tile to hide the x reload bandwidth.
"""

import numpy as np
import ml_dtypes

import concourse.mybir as mybir
import concourse.tile as tile
from concourse import bacc
from concourse.bass_utils import run_bass_kernel_spmd

B, S, IN_F, OUT_F = 4, 512, 4096, 11008
T = B * S                      # 2048
NCORES = 8
O_PAD = 11264                  # next multiple of 8*128 above OUT_F
O_PER = O_PAD // NCORES        # 1408
M_TILES = O_PER // 128         # 11
K_TILES = IN_F // 128          # 32
N_TILE = 512

VARIANT = "fp32r"              # "fp32r" | "bf16"

_NC = {}


def _emit_mgroup(nc, wp, pp, op, x_sb, wq, sb_sb, outT, group, half, n_tiles,
                 t_off, mm_dt):
    """Emit one group of m-tiles: weight loads, k-interleaved matmuls,
    fused scale+bias epilogue, and output stores."""
    w_sbs = {}
    for m in group:
        w_sb = wp.tile([128, K_TILES, 128], mm_dt, tag="w",
                       name=f"w_{half}_{m}")
        nc.gpsimd.dma_start(
            w_sb[:],
            wq[:, m * K_TILES * 128:(m + 1) * K_TILES * 128]
            .rearrange("p (ko o) -> p ko o", o=128),
        )
        w_sbs[m] = w_sb
    psum = {
        (m, n): pp.tile([128, N_TILE], mybir.dt.float32, tag="psum",
                        name=f"psum_{half}_{m}_{n}")
        for m in group for n in range(n_tiles)
    }
    for k in range(K_TILES):
        for m in group:
            for n in range(n_tiles):
                nc.tensor.matmul(
                    psum[m, n][:],
                    w_sbs[m][:, k, :],
                    x_sb[k][:, n * N_TILE:(n + 1) * N_TILE],
                    start=(k == 0),
                    stop=(k == K_TILES - 1),
                )
    for m in group:
        for n in range(n_tiles):
            o_sb = op.tile([128, N_TILE], mybir.dt.float32, tag="o",
                           name=f"o_{half}_{m}_{n}")
            nc.scalar.activation(
                o_sb[:], psum[m, n][:],
                mybir.ActivationFunctionType.Identity,
                bias=sb_sb[:, M_TILES + m:M_TILES + m + 1],
                scale=sb_sb[:, m:m + 1],
            )
            nc.sync.dma_start(
                outT[m * 128:(m + 1) * 128,
                     t_off + n * N_TILE:t_off + (n + 1) * N_TILE],
                o_sb[:],
            )


def _build(variant):
    fp32r = variant == "fp32r"
    mm_dt = mybir.dt.float32r if fp32r else mybir.dt.bfloat16
    x_dram_dt = mybir.dt.float32r if fp32r else mybir.dt.bfloat16
    n_halves = 2 if fp32r else 1
    t_half = T // n_halves

    nc = bacc.Bacc("TRN2", target_bir_lowering=False, debug=False,
                   num_devices=NCORES)
    xT = nc.dram_tensor("xT", [IN_F, T], x_dram_dt, kind="ExternalInput")
    wq = nc.dram_tensor("wq", [128, M_TILES * K_TILES * 128], mybir.dt.int8,
                        kind="ExternalInput")
    sb = nc.dram_tensor("sb", [128, 2 * M_TILES], mybir.dt.float32,
                        kind="ExternalInput")
    outT = nc.dram_tensor("outT", [O_PER, T], mybir.dt.float32,
                          kind="ExternalOutput")

    n_tiles = t_half // N_TILE
    # first group of each half sized so the PE has >= one x-DMA worth of
    # matmul work per k-tile; the rest go singly (psum-bank friendly)
    head_group = 8 // n_tiles if fp32r else 2
    groups = [list(range(head_group))] + [[m] for m in range(head_group,
                                                             M_TILES)]

    with tile.TileContext(nc) as tc:
        with (
            tc.tile_pool(name="xp", bufs=1) as xp,
            tc.tile_pool(name="wp", bufs=2) as wp,
            tc.tile_pool(name="op", bufs=4) as op,
            tc.tile_pool(name="cp", bufs=1) as cp,
            tc.tile_pool(name="ps", bufs=8, space="PSUM") as pp,
        ):
            # scale in columns [0:M_TILES), bias in [M_TILES:2*M_TILES)
            sb_sb = cp.tile([128, 2 * M_TILES], mybir.dt.float32, tag="sb")
            nc.sync.dma_start(sb_sb[:], sb[:, :])

            for half in range(n_halves):
                x_sb = []
                for k in range(K_TILES):
                    t = xp.tile([128, t_half], mm_dt, tag=f"x{k}",
                                name=f"x_{half}_{k}")
                    nc.sync.dma_start(
                        t[:], xT[k * 128:(k + 1) * 128,
                                 half * t_half:(half + 1) * t_half])
                    x_sb.append(t)
                for group in groups:
                    _emit_mgroup(nc, wp, pp, op, x_sb, wq, sb_sb, outT,
                                 group, half, n_tiles, half * t_half, mm_dt)
    nc.compile()
    return nc


def _get_nc():
    if VARIANT not in _NC:
        _NC[VARIANT] = _build(VARIANT)
    return _NC[VARIANT]


def _prepare_in_maps(x, W_q, weight_scale, bias):
    x = np.asarray(x, dtype=np.float32)
    W_q = np.asarray(W_q).astype(np.int8, copy=False)
    weight_scale = np.asarray(weight_scale, dtype=np.float32).reshape(-1)
    bias = np.asarray(bias, dtype=np.float32).reshape(-1)

    xT = np.ascontiguousarray(x.reshape(T, IN_F).T)
    if VARIANT == "bf16":
        xT = xT.astype(ml_dtypes.bfloat16)

    Wp = np.zeros((O_PAD, IN_F), dtype=np.int8)
    Wp[:OUT_F] = W_q
    scp = np.zeros((O_PAD,), np.float32)
    scp[:OUT_F] = weight_scale
    bsp = np.zeros((O_PAD,), np.float32)
    bsp[:OUT_F] = bias

    # per-core SBUF weight image [ki, m, ko, o]
    Wimg = Wp.reshape(NCORES, M_TILES, 128, K_TILES, 128)  # (c, m, o, ko, ki)
    Wimg = np.ascontiguousarray(Wimg.transpose(0, 4, 1, 3, 2)).reshape(
        NCORES, 128, M_TILES * K_TILES * 128)
    sc = scp.reshape(NCORES, M_TILES, 128).transpose(0, 2, 1)   # (c, p, m)
    bs = bsp.reshape(NCORES, M_TILES, 128).transpose(0, 2, 1)
    sbimg = np.ascontiguousarray(np.concatenate([sc, bs], axis=2),
                                 dtype=np.float32)               # (c, 128, 2M)

    return [{"xT": xT, "wq": Wimg[c], "sb": sbimg[c]} for c in range(NCORES)]


def _gather(results):
    outT = np.concatenate([r["outT"] for r in results], axis=0)  # [O_PAD, T]
    out = outT[:OUT_F].T.reshape(B, S, OUT_F)
    return np.ascontiguousarray(out, dtype=np.float32)


def _run(x, W_q, weight_scale, bias, **run_kwargs):
    nc = _get_nc()
    in_maps = _prepare_in_maps(x, W_q, weight_scale, bias)
    res = run_bass_kernel_spmd(nc, in_maps, core_ids=list(range(NCORES)),
                               **run_kwargs)
    return _gather(res.results), res


def kernel(x, W_q, weight_scale, bias):
    out, _ = _run(x, W_q, weight_scale, bias)
    return out


# revision 4
# speedup vs baseline: 1.1738x; 1.1738x over previous
"""AWQ int8 linear (x @ (W_q * scale).T + bias) on 8 Trainium2 NeuronCores.

Column-parallel sharding: out_features padded 11008 -> 11264 and split 1408
per core; x is replicated. Per core computes outT[o, t] = scale[o] *
sum_i W_q[o, i] * x[t, i] + bias[o], PE accumulating in fp32 PSUM, with a
per-partition scale+bias fused into one ScalarE activation.

Precision variants:
 - "fp32r": PE fp32r mode (~1e-4 rel err). x (fp32) is too big for SBUF, so
   the token dim is processed in 2 resident halves. All weights stay
   SBUF-resident as int8 (44 KB/partition) and are dequantized to fp32r
   tiles on the (otherwise idle) Vector/Scalar engines right before use.
 - "bf16": x host-cast to bf16 (~1.7e-3 rel err), whole x resident, weights
   streamed once with an int8->bf16 cast inside the DMA.

Host-side layout prep: x is transposed to xT [IN_F, T] so the contraction
dim lands on SBUF partitions; per-core weights are pre-arranged to the exact
SBUF image [ki=128, m, ko, o] so every DMA is contiguous per partition.
The first m-tiles of each half are emitted k-interleaved as a group so the
PE has at least one x-DMA worth of matmul work per arriving x k-tile.
"""

import numpy as np
import ml_dtypes

import concourse.mybir as mybir
import concourse.tile as tile
from concourse import bacc
from concourse.bass_utils import run_bass_kernel_spmd

B, S, IN_F, OUT_F = 4, 512, 4096, 11008
T = B * S                      # 2048
NCORES = 8
O_PAD = 11264                  # next multiple of 8*128 above OUT_F
O_PER = O_PAD // NCORES        # 1408
M_TILES = O_PER // 128         # 11
K_TILES = IN_F // 128          # 32
N_TILE = 512

VARIANT = "fp32r"              # "fp32r" | "bf16"

_NC = {}


def _build_fp32r(nc, tc, xT, wq, sb, outT):
    f32r = mybir.dt.float32r
    n_halves, t_half = 2, T // 2
    n_tiles = t_half // N_TILE          # 2
    head = [0, 1, 2, 3]                 # k-interleaved first group per half
    groups = [head] + [[m] for m in range(len(head), M_TILES)]

    with (
        tc.tile_pool(name="xp", bufs=1) as xp,
        tc.tile_pool(name="w8p", bufs=1) as w8p,
        tc.tile_pool(name="wstp", bufs=12) as wstp,
        tc.tile_pool(name="op", bufs=3) as op,
        tc.tile_pool(name="cp", bufs=1) as cp,
        tc.tile_pool(name="ps", bufs=8, space="PSUM") as pp,
    ):
        sb_sb = cp.tile([128, 2 * M_TILES], mybir.dt.float32, tag="sb")
        nc.sync.dma_start(sb_sb[:], sb[:, :])

        # all weights resident as int8 [ki, ko, o] per m-tile; loaded once
        # on the scalar HWDGE ring (parallel to x loads on the sync ring)
        w8 = []
        for m in range(M_TILES):
            t = w8p.tile([128, K_TILES, 128], mybir.dt.int8, tag=f"w8_{m}")
            nc.scalar.dma_start(
                t[:],
                wq[:, m * K_TILES * 128:(m + 1) * K_TILES * 128]
                .rearrange("p (ko o) -> p ko o", o=128))
            w8.append(t)

        def stage(half, m, k):
            """dequant w8[m][:, k, :] (int8) -> fp32r tile on DVE/ACT"""
            wst = wstp.tile([128, 128], f32r, tag="wst",
                            name=f"wst_{half}_{m}_{k}")
            if k % 2 == 0:
                nc.vector.tensor_copy(wst[:], w8[m][:, k, :])
            else:
                nc.scalar.copy(wst[:], w8[m][:, k, :])
            return wst

        for half in range(n_halves):
            x_sb = []
            for k in range(K_TILES):
                t = xp.tile([128, t_half], f32r, tag=f"x{k}",
                            name=f"x_{half}_{k}")
                nc.sync.dma_start(
                    t[:], xT[k * 128:(k + 1) * 128,
                             half * t_half:(half + 1) * t_half])
                x_sb.append(t)

            for group in groups:
                psum = {
                    (m, n): pp.tile([128, N_TILE], mybir.dt.float32,
                                    tag="psum", name=f"ps_{half}_{m}_{n}")
                    for m in group for n in range(n_tiles)
                }
                for k in range(K_TILES):
                    for m in group:
                        wst = stage(half, m, k)
                        for n in range(n_tiles):
                            nc.tensor.matmul(
                                psum[m, n][:],
                                wst[:],
                                x_sb[k][:, n * N_TILE:(n + 1) * N_TILE],
                                start=(k == 0),
                                stop=(k == K_TILES - 1),
                            )
                for m in group:
                    for n in range(n_tiles):
                        o_sb = op.tile([128, N_TILE], mybir.dt.float32,
                                       tag="o", name=f"o_{half}_{m}_{n}")
                        nc.scalar.activation(
                            o_sb[:], psum[m, n][:],
                            mybir.ActivationFunctionType.Identity,
                            bias=sb_sb[:, M_TILES + m:M_TILES + m + 1],
                            scale=sb_sb[:, m:m + 1],
                        )
                        nc.sync.dma_start(
                            outT[m * 128:(m + 1) * 128,
                                 half * t_half + n * N_TILE:
                                 half * t_half + (n + 1) * N_TILE],
                            o_sb[:],
                        )


def _build_bf16(nc, tc, xT, wq, sb, outT):
    bf16 = mybir.dt.bfloat16
    n_tiles = T // N_TILE               # 4
    head = [0, 1]
    groups = [head] + [[m] for m in range(len(head), M_TILES)]

    with (
        tc.tile_pool(name="xp", bufs=1) as xp,
        tc.tile_pool(name="wp", bufs=4) as wp,
        tc.tile_pool(name="op", bufs=3) as op,
        tc.tile_pool(name="cp", bufs=1) as cp,
        tc.tile_pool(name="ps", bufs=8, space="PSUM") as pp,
    ):
        sb_sb = cp.tile([128, 2 * M_TILES], mybir.dt.float32, tag="sb")
        nc.sync.dma_start(sb_sb[:], sb[:, :])

        x_sb = []
        for k in range(K_TILES):
            t = xp.tile([128, T], bf16, tag=f"x{k}")
            nc.sync.dma_start(t[:], xT[k * 128:(k + 1) * 128, :])
            x_sb.append(t)

        for group in groups:
            w_sbs = {}
            for m in group:
                w_sb = wp.tile([128, K_TILES, 128], bf16, tag="w",
                               name=f"w_{m}")
                nc.gpsimd.dma_start(
                    w_sb[:],
                    wq[:, m * K_TILES * 128:(m + 1) * K_TILES * 128]
                    .rearrange("p (ko o) -> p ko o", o=128))
                w_sbs[m] = w_sb
            psum = {
                (m, n): pp.tile([128, N_TILE], mybir.dt.float32, tag="psum",
                                name=f"ps_{m}_{n}")
                for m in group for n in range(n_tiles)
            }
            for k in range(K_TILES):
                for m in group:
                    for n in range(n_tiles):
                        nc.tensor.matmul(
                            psum[m, n][:],
                            w_sbs[m][:, k, :],
                            x_sb[k][:, n * N_TILE:(n + 1) * N_TILE],
                            start=(k == 0),
                            stop=(k == K_TILES - 1),
                        )
            for m in group:
                for n in range(n_tiles):
                    o_sb = op.tile([128, N_TILE], mybir.dt.float32, tag="o",
                                   name=f"o_{m}_{n}")
                    nc.scalar.activation(
                        o_sb[:], psum[m, n][:],
                        mybir.ActivationFunctionType.Identity,
                        bias=sb_sb[:, M_TILES + m:M_TILES + m + 1],
                        scale=sb_sb[:, m:m + 1],
                    )
                    nc.sync.dma_start(
                        outT[m * 128:(m + 1) * 128,
                             n * N_TILE:(n + 1) * N_TILE],
                        o_sb[:],
                    )


def _build(variant):
    fp32r = variant == "fp32r"
    x_dram_dt = mybir.dt.float32r if fp32r else mybir.dt.bfloat16

    nc = bacc.Bacc("TRN2", target_bir_lowering=False, debug=False,
                   num_devices=NCORES)
    xT = nc.dram_tensor("xT", [IN_F, T], x_dram_dt, kind="ExternalInput")
    wq = nc.dram_tensor("wq", [128, M_TILES * K_TILES * 128], mybir.dt.int8,
                        kind="ExternalInput")
    sb = nc.dram_tensor("sb", [128, 2 * M_TILES], mybir.dt.float32,
                        kind="ExternalInput")
    outT = nc.dram_tensor("outT", [O_PER, T], mybir.dt.float32,
                          kind="ExternalOutput")

    with tile.TileContext(nc) as tc:
        if fp32r:
            _build_fp32r(nc, tc, xT, wq, sb, outT)
        else:
            _build_bf16(nc, tc, xT, wq, sb, outT)
    nc.compile()
    return nc


def _get_nc():
    if VARIANT not in _NC:
        _NC[VARIANT] = _build(VARIANT)
    return _NC[VARIANT]


def _prepare_in_maps(x, W_q, weight_scale, bias):
    x = np.asarray(x, dtype=np.float32)
    W_q = np.asarray(W_q).astype(np.int8, copy=False)
    weight_scale = np.asarray(weight_scale, dtype=np.float32).reshape(-1)
    bias = np.asarray(bias, dtype=np.float32).reshape(-1)

    xT = np.ascontiguousarray(x.reshape(T, IN_F).T)
    if VARIANT == "bf16":
        xT = xT.astype(ml_dtypes.bfloat16)

    Wp = np.zeros((O_PAD, IN_F), dtype=np.int8)
    Wp[:OUT_F] = W_q
    scp = np.zeros((O_PAD,), np.float32)
    scp[:OUT_F] = weight_scale
    bsp = np.zeros((O_PAD,), np.float32)
    bsp[:OUT_F] = bias

    # per-core SBUF weight image [ki, m, ko, o]
    Wimg = Wp.reshape(NCORES, M_TILES, 128, K_TILES, 128)  # (c, m, o, ko, ki)
    Wimg = np.ascontiguousarray(Wimg.transpose(0, 4, 1, 3, 2)).reshape(
        NCORES, 128, M_TILES * K_TILES * 128)
    sc = scp.reshape(NCORES, M_TILES, 128).transpose(0, 2, 1)   # (c, p, m)
    bs = bsp.reshape(NCORES, M_TILES, 128).transpose(0, 2, 1)
    sbimg = np.ascontiguousarray(np.concatenate([sc, bs], axis=2),
                                 dtype=np.float32)               # (c, 128, 2M)

    return [{"xT": xT, "wq": Wimg[c], "sb": sbimg[c]} for c in range(NCORES)]


def _gather(results):
    outT = np.concatenate([r["outT"] for r in results], axis=0)  # [O_PAD, T]
    out = outT[:OUT_F].T.reshape(B, S, OUT_F)
    return np.ascontiguousarray(out, dtype=np.float32)


def _run(x, W_q, weight_scale, bias, **run_kwargs):
    nc = _get_nc()
    in_maps = _prepare_in_maps(x, W_q, weight_scale, bias)
    res = run_bass_kernel_spmd(nc, in_maps, core_ids=list(range(NCORES)),
                               **run_kwargs)
    return _gather(res.results), res


def kernel(x, W_q, weight_scale, bias):
    out, _ = _run(x, W_q, weight_scale, bias)
    return out
